# revision 2
# baseline (speedup 1.0000x reference)
# Bass/Tile TRN2 kernel for nn_Conv1D_style: out = ((x * (cluster@style_L)) @ weight) * (cluster@style_R)
#
# Sharding: data-parallel over the batch dim. Each of the 8 cores gets a
# 1024-row slice of x/cluster and a full (replicated) weight/style_L/style_R.
#
# Per-core plan (M=1024 batch, K=4096 din, N=4096 dout):
#   Phase A: for each k-tile (128 din rows):
#     tmpLT[k] = style_L[:, kslice].T @ clusterT          (PE, K=64 padded to 128)
#     aT[k]    = xT[k] * tmpLT[k]   -> bf16, resident in SBUF (8 MiB total)
#   Main: for n (8 x 512 dout cols):  stream W[:, nslice] (bf16)
#     for m (8 x 128 batch rows):
#       tmpR  = clusterT[:, mslice].T @ style_R[:, nslice] (PE) -> SBUF via copy
#       psum  = sum_k aT[k][:, mslice].T @ W[k, nslice]    (32 accumulating MMs)
#       out   = psum * tmpR   (DVE)  -> DMA to DRAM (natural [batch, dout] layout)
#
# All matmuls are bf16 inputs with fp32 PSUM accumulation.

import numpy as np
import ml_dtypes

B, DIN, DOUT, NCL = 8192, 4096, 4096, 64
NCORES = 8
MB = B // NCORES          # batch rows per core
P = 128
NT = 512                  # n tile (dout cols per matmul)
KT = DIN // P             # 32 k tiles
MT = MB // P              # 8 m tiles
NTS = DOUT // NT          # 8 n tiles

_CACHE = {}
LAST = {}                 # exposes the most recent BassKernelResults for test harnesses


def _build_program():
    import concourse.bass as bass
    import concourse.bacc as bacc
    import concourse.mybir as mybir
    import concourse.tile as tile

    bf16 = mybir.dt.bfloat16
    f32 = mybir.dt.float32

    nc = bacc.Bacc(None, target_bir_lowering=False, debug=False)

    xT_d = nc.declare_dram_parameter("xT", [DIN, MB], bf16, isOutput=False)
    clT_d = nc.declare_dram_parameter("clusterT", [P, MB], bf16, isOutput=False)
    w_d = nc.declare_dram_parameter("weight", [DIN, DOUT], bf16, isOutput=False)
    sL_d = nc.declare_dram_parameter("style_L", [P, DIN], bf16, isOutput=False)
    sR_d = nc.declare_dram_parameter("style_R", [P, DOUT], bf16, isOutput=False)
    out_d = nc.declare_dram_parameter("out", [MB, DOUT], f32, isOutput=True)

    with tile.TileContext(nc) as tc:
        with (
            tc.tile_pool(name="const", bufs=1) as const_pool,
            tc.tile_pool(name="atp", bufs=1) as at_pool,
            tc.tile_pool(name="wp", bufs=2) as w_pool,
            tc.tile_pool(name="xp", bufs=3) as x_pool,
            tc.tile_pool(name="evp", bufs=3) as ev_pool,
            tc.tile_pool(name="pyp", bufs=2, space="PSUM") as py_pool,
            tc.tile_pool(name="prp", bufs=2, space="PSUM") as pr_pool,
            tc.tile_pool(name="plp", bufs=2, space="PSUM") as pl_pool,
        ):
            # ---- constants ----
            clT = const_pool.tile([P, MB], bf16, name="clT")
            sL = const_pool.tile([P, DIN], bf16, name="sL")
            sR = const_pool.tile([P, DOUT], bf16, name="sR")
            nc.sync.dma_start(clT[:], clT_d[:])
            nc.sync.dma_start(sL[:], sL_d[:])
            nc.sync.dma_start(sR[:], sR_d[:])

            # ---- Phase A: aT[k] = xT[k] * (style_L[:, k].T @ clusterT) ----
            at_tiles = []
            for k in range(KT):
                xk = x_pool.tile([P, MB], bf16, name=f"xk{k}", tag="xk")
                nc.sync.dma_start(xk[:], xT_d[k * P:(k + 1) * P, :])
                pl = pl_pool.tile([P, MB], f32, name=f"pl{k}", tag="pl")
                for j in range(MB // NT):
                    nc.tensor.matmul(
                        pl[:, j * NT:(j + 1) * NT],
                        sL[:, k * P:(k + 1) * P],
                        clT[:, j * NT:(j + 1) * NT],
                        start=True, stop=True,
                    )
                at_k = at_pool.tile([P, MB], bf16, name=f"at{k}", tag=f"at{k}")
                nc.vector.tensor_mul(out=at_k[:], in0=xk[:], in1=pl[:])
                at_tiles.append(at_k)

            # ---- Main loop ----
            for n in range(NTS):
                wk = []
                for k in range(KT):
                    wt = w_pool.tile([P, NT], bf16, name=f"w{n}_{k}", tag=f"w{k}")
                    nc.sync.dma_start(
                        wt[:], w_d[k * P:(k + 1) * P, n * NT:(n + 1) * NT]
                    )
                    wk.append(wt)
                for m in range(MT):
                    pr = pr_pool.tile([P, NT], f32, name=f"pr{n}_{m}", tag="pr")
                    nc.tensor.matmul(
                        pr[:],
                        clT[:, m * P:(m + 1) * P],
                        sR[:, n * NT:(n + 1) * NT],
                        start=True, stop=True,
                    )
                    tr = ev_pool.tile([P, NT], f32, name=f"tr{n}_{m}", tag="tr")
                    nc.any.tensor_copy(out=tr[:], in_=pr[:])

                    py = py_pool.tile([P, NT], f32, name=f"py{n}_{m}", tag="py")
                    for k in range(KT):
                        nc.tensor.matmul(
                            py[:],
                            at_tiles[k][:, m * P:(m + 1) * P],
                            wk[k][:],
                            start=(k == 0), stop=(k == KT - 1),
                        )
                    ot = ev_pool.tile([P, NT], f32, name=f"ot{n}_{m}", tag="ot")
                    nc.vector.tensor_mul(out=ot[:], in0=py[:], in1=tr[:])
                    nc.sync.dma_start(
                        out_d[m * P:(m + 1) * P, n * NT:(n + 1) * NT], ot[:]
                    )

    nc.finalize()
    return nc


def _get_program():
    if "nc" not in _CACHE:
        _CACHE["nc"] = _build_program()
    return _CACHE["nc"]


def kernel(x, cluster, weight, style_L, style_R):
    from concourse.bass_utils import run_bass_kernel_spmd

    nc = _get_program()
    bf16 = ml_dtypes.bfloat16

    w_bf = np.asarray(weight, dtype=np.float32).astype(bf16)
    sL = np.zeros((P, DIN), dtype=bf16)
    sL[:NCL] = np.asarray(style_L, dtype=np.float32).astype(bf16)
    sR = np.zeros((P, DOUT), dtype=bf16)
    sR[:NCL] = np.asarray(style_R, dtype=np.float32).astype(bf16)

    in_maps = []
    for c in range(NCORES):
        xs = np.asarray(x[c * MB:(c + 1) * MB], dtype=np.float32)
        xT = np.ascontiguousarray(xs.T).astype(bf16)
        clT = np.zeros((P, MB), dtype=bf16)
        clT[:NCL] = np.ascontiguousarray(
            np.asarray(cluster[c * MB:(c + 1) * MB], dtype=np.float32).T
        ).astype(bf16)
        in_maps.append(
            {"xT": xT, "clusterT": clT, "weight": w_bf, "style_L": sL, "style_R": sR}
        )

    res = run_bass_kernel_spmd(nc, in_maps, list(range(NCORES)))
    LAST["results"] = res
    LAST["in_maps"] = in_maps
    out = np.concatenate(
        [np.asarray(res.results[c]["out"], dtype=np.float32) for c in range(NCORES)],
        axis=0,
    )
    return out
